# revision 13
# baseline (speedup 1.0000x reference)
"""Trainium2 Bass kernel for nn_Clustering_28389733826971 (vq_codebook).

Three uniform SPMD launches over 8 cores (core c owns batch b=c); the
between-launch glue on the host is pure indexing (zero FLOPs):
  A: maxpool K/V, transpose pooled-K to KpT = Kp.reshape(912,128).T
  B: cluster-q MLP restructured as residue-class sparse matmuls over the
     unfold's structural zeros, softmax stats, first-argmax onehot, cluster
     centers, ln(softplus(std)) partial sums for the loss
  C: Wp projection of the 10 cc blocks this core's attention needs, then
     scores/softmax/context per head

Math notes (host-verified vs reference to ~3e-7 rel):
 - mean over classes of a softmax row is exactly 1/nc, so x == mu; the
   ck-MLP (Wk*) never affects the output; log_prob = -ln(sigma)-ln(2pi)/2;
   the CE term is the data-independent constant 22.8*ln(228).
 - K_unf rows are 512-wide windows of a 96.5%-structural-zero buffer; the
   nonzeros of (row class rho = r mod 25, shift s) form the contiguous
   partition slice KpT[b-s][g0:g0+nf].
"""

import numpy as np

import concourse.bass as bass
import concourse.mybir as mybir
import concourse.tile as tile
from concourse.bass_utils import run_bass_kernel_spmd
from concourse.masks import make_identity
from concourse.vector_clock import ScopedClock, VectorClock

NCL, DM, H, DK, B, L = 10, 512, 8, 64, 8, 2048
U, LK, G = 100, 228, 512
AL = mybir.AluOpType
AF = mybir.ActivationFunctionType
F32 = mybir.dt.float32
X = mybir.AxisListType.X


def _patch_tile_drain():
    """This walrus build allows only one sync-wait per CTRL (Drain) inst;
    split the TileContext exit drain's waits across per-proc drains."""
    if getattr(tile.TileContext, "_drain_patched", False):
        return

    def _split(self, tick_clock, wait_clock):
        gc = tick_clock.global_clock
        n = len(gc)
        for p in range(n):
            if gc[p] <= 0:
                continue
            vec = [0] * n
            vec[p] = gc[p]
            d = self.nc.sync.drain()
            wait_clock.add_sem_waits(d.ins, ScopedClock({None: VectorClock(vec)}))
        self.nc.sync.drain()
        self.nc.all_engine_barrier()
        assert self.sems is not None
        popped = self.nc._tile_sem_poison_stack.pop()
        assert popped is self._sem_poison
        self.nc.clear_and_free_semaphores(list(self.sems.allocated().values()))
        self.nc.all_engine_barrier()

    tile.TileContext._drain_and_barrier = _split
    tile.TileContext._drain_patched = True
    # This walrus build accepts at most ONE sync-wait per instruction.
    # Rewrite the serialized BIR before compile: extra waits move onto
    # fresh single-wait Nop instructions spliced just before the owner,
    # on the same engine (same-sequencer program order preserves semantics).
    import json as _json
    import concourse.bass2jax as _b2j
    if not getattr(_b2j, "_wsplit_patched", False):
        _orig_compile = _b2j.compile_bir_kernel

        def _split_waits_compile(bir_json, tmpdir, neff_name="file.neff"):
            m = _json.loads(bir_json)
            ctr = 0
            for fn in m["functions"]:
                for blk in fn["blocks"]:
                    out = []
                    for ins in blk["instructions"]:
                        si = ins.get("sync_info") or {}
                        ow = si.get("on_wait") or []
                        if len(ow) > 1:
                            for w in ow[:-1]:
                                ctr += 1
                                nop = {
                                    "opcode": "Drain", "engine": ins["engine"],
                                    "name": f"WSPLIT-{ctr}", "ins": [],
                                    "outs": [], "is_reset_sema": False,
                                    "sync_info": {"on_wait": [w],
                                                  "on_update": []},
                                }
                                if "debug" in ins:
                                    nop["debug"] = ins["debug"]
                                out.append(nop)
                            si["on_wait"] = [ow[-1]]
                        out.append(ins)
                    blk["instructions"] = out
            return _orig_compile(_json.dumps(m).encode(), tmpdir, neff_name)

        _b2j.compile_bir_kernel = _split_waits_compile
        _b2j._wsplit_patched = True


def _slot(s, rho):
    ustar = 99 - s
    phi = (ustar - 512 * rho) % 100
    fs = list(range(phi, 512, 100))
    g0 = (512 * rho + fs[0] - ustar) // 100
    return fs, g0


def _build_a():
    """Pool K,V; emit Kp-transpose, pooled V."""
    nc = bass.Bass()
    kin = nc.declare_dram_parameter("kin", [G, L], F32, isOutput=False)
    vin = nc.declare_dram_parameter("vin", [G, L], F32, isOutput=False)
    kpt = nc.declare_dram_parameter("kpt", [128, 912], F32, isOutput=True)
    vp = nc.declare_dram_parameter("vp", [G, LK], F32, isOutput=True)
    kp_d = nc.dram_tensor("kp_d", [G, LK], F32)

    with tile.TileContext(nc) as tc:
        with tc.tile_pool(name="c0", bufs=1) as const, \
             tc.tile_pool(name="sb", bufs=3) as sb, \
             tc.tile_pool(name="ps", bufs=2, space="PSUM") as ps:
            ident = const.tile([128, 128], F32)
            make_identity(nc, ident[:])
            for src, dst in ((kin, kp_d), (vin, vp)):
                for t in range(4):
                    xt = sb.tile([128, L], F32, tag="pin")
                    nc.sync.dma_start(out=xt[:], in_=src[t * 128:(t + 1) * 128, :])
                    ot = sb.tile([128, LK], F32, tag="pout")
                    nc.vector.tensor_reduce(ot[:, 0:1],
                                            xt[:, 0:5].rearrange("p (a b) -> p a b", a=1),
                                            axis=X, op=AL.max)
                    nc.vector.tensor_reduce(ot[:, 1:LK],
                                            xt[:, 5:L].rearrange("p (a b) -> p a b", b=9),
                                            axis=X, op=AL.max)
                    nc.sync.dma_start(out=dst[t * 128:(t + 1) * 128, :], in_=ot[:])
            kp_flat = kp_d.ap().rearrange("(a g) t -> a (g t)", a=1)
            for t in range(8):
                rows = 128 if t < 7 else 16
                chunk = sb.tile([128, 128], F32, tag="kpc")
                nc.sync.dma_start(
                    out=chunk[:rows, :],
                    in_=kp_flat[0, t * 16384:t * 16384 + rows * 128].rearrange(
                        "(r c) -> r c", c=128))
                tp = ps.tile([128, 128], F32, tag="kptp")
                nc.tensor.transpose(tp[:, :rows], chunk[:rows, :], ident[:rows, :rows])
                out_sb = sb.tile([128, 128], F32, tag="kpo")
                nc.vector.tensor_copy(out_sb[:, :rows], tp[:, :rows])
                nc.sync.dma_start(out=kpt[:, t * 128:t * 128 + rows], in_=out_sb[:, :rows])
    return nc


def _build_b():
    """q-MLP + stats + centers + loss partial. All per-core variation is in
    the kptjl input (slice s is KpT[b-s], zeros when b-s < 1)."""
    nc = bass.Bass()
    kjl = nc.declare_dram_parameter("kjl", [128, 7 * 912], F32, isOutput=False)
    w1p = nc.declare_dram_parameter("w1p", [128, 175 * 40], F32, isOutput=False)
    w2a = nc.declare_dram_parameter("w2a", [41, NCL], F32, isOutput=False)
    bq1 = nc.declare_dram_parameter("bq1", [40, 1], F32, isOutput=False)
    cex = nc.declare_dram_parameter("cex", [NCL, LK * NCL], F32, isOutput=True)
    lossp = nc.declare_dram_parameter("lossp", [1, 1], F32, isOutput=True)

    with tile.TileContext(nc) as tc:
        with tc.tile_pool(name="c0", bufs=1) as const, \
             tc.tile_pool(name="pers", bufs=1) as pers, \
             tc.tile_pool(name="sb", bufs=3) as sb, \
             tc.tile_pool(name="sb2", bufs=2) as sb2, \
             tc.tile_pool(name="ps", bufs=2, space="PSUM") as ps, \
             tc.tile_pool(name="ps4", bufs=2, space="PSUM") as ps4:
            ident = const.tile([128, 128], F32)
            make_identity(nc, ident[:])
            iota99 = const.tile([114, 40], F32)
            nc.gpsimd.iota(iota99[:], pattern=[[0, 4], [1, 10]], base=-99,
                           channel_multiplier=0,
                           allow_small_or_imprecise_dtypes=True)
            kptj = pers.tile([128, 7, 912], F32)
            nc.sync.dma_start(out=kptj[:], in_=kjl[:])
            w1sb = const.tile([128, 175 * 40], F32)
            nc.sync.dma_start(out=w1sb[:], in_=w1p[:])
            w2sb = const.tile([41, NCL], F32)
            nc.sync.dma_start(out=w2sb[:], in_=w2a[:])
            bq1sb = const.tile([40, 1], F32)
            nc.sync.dma_start(out=bq1sb[:], in_=bq1[:])

            cqn_all = [pers.tile([114, U, NCL], F32, tag=f"cq{i}", name=f"cqn{i}") for i in range(2)]
            oh_all = [pers.tile([114, U, NCL], F32, tag=f"oh{i}", name=f"ohall{i}") for i in range(2)]
            lnacc = [pers.tile([114, 4], F32, tag=f"ln{i}", name=f"lnacc{i}") for i in range(2)]
            for lh in range(2):
                nc.vector.memset(lnacc[lh][:], 0.0)
            cexsb = pers.tile([11, LK, NCL], F32)
            nc.vector.memset(cexsb[:], 1.0)

            for rho in range(25):
                g1 = sb.tile([41, 912], F32, tag="g1")
                nc.vector.memset(g1[:], 1.0)
                for ph in range(2):
                    h1 = ps.tile([40, 456], F32, tag="h1")
                    for s in range(7):
                        rhs = kptj[:, s, :].rearrange(
                            "p (l w) -> p w l", w=4)[:, 2 * ph:2 * ph + 2, :]
                        nc.tensor.matmul(
                            h1[:],
                            w1sb[:, (s * 25 + rho) * 40:(s * 25 + rho + 1) * 40],
                            rhs, start=(s == 0), stop=(s == 6))
                    nc.scalar.activation(g1[0:40, ph * 456:(ph + 1) * 456], h1[:],
                                         AF.Gelu, bias=bq1sb[:])
                for lh in range(2):
                    h2 = ps4.tile([114, 40], F32, tag="h2")
                    for w in range(4):
                        nc.tensor.matmul(
                            h2[:, w * 10:(w + 1) * 10],
                            g1[:, w * LK + lh * 114:w * LK + lh * 114 + 114],
                            w2sb[:], start=True, stop=True)
                    est = sb.tile([114, 4, NCL], F32, tag="est")
                    nc.scalar.activation(
                        est[:], h2.rearrange("p (w c) -> p w c", c=10), AF.Exp)
                    s1 = sb.tile([114, 4], F32, tag="s1")
                    nc.vector.tensor_reduce(s1[:], est[:], axis=X, op=AL.add)
                    rc = sb.tile([114, 4], F32, tag="rc")
                    nc.vector.reciprocal(rc[:], s1[:])
                    nc.vector.tensor_mul(
                        cqn_all[lh][:, rho::25, :], est[:],
                        rc[:].rearrange("p (w a) -> p w a", a=1).broadcast_to([114, 4, 10]))
                    e2 = sb.tile([114, 4, NCL], F32, tag="e2")
                    nc.scalar.square(e2[:], est[:])
                    s2 = sb.tile([114, 4], F32, tag="s2")
                    nc.vector.tensor_reduce(s2[:], e2[:], axis=X, op=AL.add)
                    mx = sb.tile([114, 4], F32, tag="mx")
                    nc.vector.tensor_reduce(mx[:], est[:], axis=X, op=AL.max)
                    eq = sb.tile([114, 4, NCL], F32, tag="eq")
                    nc.vector.tensor_tensor(
                        eq[:], est[:],
                        mx[:].rearrange("p (w a) -> p w a", a=1).broadcast_to([114, 4, 10]),
                        op=AL.is_equal)
                    t1 = sb.tile([114, 4, NCL], F32, tag="t1")
                    nc.vector.tensor_mul(
                        t1[:], eq[:], iota99.rearrange("p (w c) -> p w c", c=10))
                    am = sb.tile([114, 4], F32, tag="am")
                    nc.vector.tensor_reduce(am[:], t1[:], axis=X, op=AL.min)
                    nc.vector.tensor_tensor(
                        oh_all[lh][:, rho::25, :],
                        iota99.rearrange("p (w c) -> p w c", c=10),
                        am[:].rearrange("p (w a) -> p w a", a=1).broadcast_to([114, 4, 10]),
                        op=AL.is_equal)
                    r2 = sb.tile([114, 4], F32, tag="r2")
                    nc.vector.tensor_mul(r2[:], rc[:], rc[:])
                    tt = sb.tile([114, 4], F32, tag="tt")
                    nc.vector.tensor_mul(tt[:], s2[:], r2[:])
                    uu = sb.tile([114, 4], F32, tag="uu")
                    nc.vector.tensor_scalar(out=uu[:], in0=tt[:], scalar1=-0.1,
                                            scalar2=0.0, op0=AL.add, op1=AL.max)
                    sd = sb.tile([114, 4], F32, tag="sd")
                    nc.scalar.activation(sd[:], uu[:], AF.Sqrt, scale=1.0 / 9.0)
                    # softplus(x) = ln(exp(x) + 1); this act table has no
                    # native Softplus
                    nc.scalar.activation(sd[:], sd[:], AF.Exp)
                    nc.scalar.activation(sd[:], sd[:], AF.Ln, bias=1.0)
                    nc.scalar.activation(sd[:], sd[:], AF.Ln)
                    nc.vector.tensor_add(lnacc[lh][:], lnacc[lh][:], sd[:])

            for lh in range(2):
                tot = sb2.tile([114, NCL], F32, tag="tot")
                nc.vector.tensor_reduce(tot[:], cqn_all[lh].transpose([0, 2, 1]),
                                        axis=X, op=AL.add)
                totp = ps.tile([NCL, 114], F32, tag="h1")
                nc.tensor.transpose(totp[:], tot[:], ident[0:114, 0:114])
                tott = sb2.tile([NCL, 114], F32, tag="tott")
                nc.vector.tensor_copy(tott[:], totp[:])
                cqt = pers.tile([U, NCL, 114], F32, tag="cqt")
                oht = pers.tile([U, NCL, 114], F32, tag="oht")
                for c in range(NCL):
                    tp1 = ps.tile([U, 114], F32, tag="h1")
                    nc.tensor.transpose(tp1[:], cqn_all[lh][:, :, c],
                                        ident[0:114, 0:114])
                    nc.vector.tensor_copy(cqt[:, c, :], tp1[:])
                    tp2 = ps.tile([U, 114], F32, tag="h1")
                    nc.tensor.transpose(tp2[:], oh_all[lh][:, :, c],
                                        ident[0:114, 0:114])
                    nc.vector.tensor_copy(oht[:, c, :], tp2[:])
                for grp in range(3):
                    nlk = 51 if grp < 2 else 12
                    spo = ps4.tile([NCL, 510], F32, tag="spo")
                    for lk in range(nlk):
                        lkg = grp * 51 + lk
                        nc.tensor.matmul(spo[:, lk * 10:(lk + 1) * 10],
                                         cqt[:, :, lkg], oht[:, :, lkg],
                                         start=True, stop=True)
                    nc.vector.scalar_tensor_tensor(
                        out=cexsb[0:NCL,
                                  lh * 114 + grp * 51:lh * 114 + grp * 51 + nlk, :],
                        in0=spo[:, 0:nlk * 10].rearrange("p (l c) -> p l c", c=10),
                        scalar=-1.0, op0=AL.mult, op1=AL.add,
                        in1=tott[:, grp * 51:grp * 51 + nlk].rearrange(
                            "p (l a) -> p l a", a=1).broadcast_to([NCL, nlk, 10]))

            nc.sync.dma_start(out=cex[:],
                              in_=cexsb[0:NCL].rearrange("p l c -> p (l c)"))
            ones114 = const.tile([114, 1], F32)
            nc.vector.memset(ones114[:], 1.0)
            lsp = ps.tile([1, 4], F32, tag="h1")
            for lh in range(2):
                nc.tensor.matmul(lsp[:], ones114[:], lnacc[lh][:],
                                 start=(lh == 0), stop=(lh == 1))
            lsums = sb.tile([1, 4], F32, tag="lsums")
            nc.vector.tensor_copy(lsums[:], lsp[:])
            lred = sb.tile([1, 1], F32, tag="lred")
            nc.vector.tensor_reduce(lred[:], lsums[:].rearrange("p (a f) -> p a f", a=1),
                                    axis=X, op=AL.add)
            nc.sync.dma_start(out=lossp[:], in_=lred[:])
    return nc


def _build_c():
    """cc blocks (centers @ Wp, gelu) + attention per head."""
    nc = bass.Bass()
    cin = nc.declare_dram_parameter("cin", [NCL, 11, LK], F32, isOutput=False)
    wpb = nc.declare_dram_parameter("wpb", [11, DM], F32, isOutput=False)
    qin = nc.declare_dram_parameter("qin", [H, L, DK], F32, isOutput=False)
    vp = nc.declare_dram_parameter("vp", [G, LK], F32, isOutput=False)
    ctx = nc.declare_dram_parameter("ctx", [H, L, DK], F32, isOutput=True)
    cc_d = nc.dram_tensor("cc_d", [NCL, LK, DM], F32)

    with tile.TileContext(nc) as tc:
        with tc.tile_pool(name="c0", bufs=1) as const, \
             tc.tile_pool(name="sb", bufs=3) as sb, \
             tc.tile_pool(name="sb2", bufs=2) as sb2, \
             tc.tile_pool(name="ps", bufs=2, space="PSUM") as ps, \
             tc.tile_pool(name="pssco", bufs=2, space="PSUM") as pssco, \
             tc.tile_pool(name="psctx", bufs=1, space="PSUM") as psctx:
            ident = const.tile([128, 128], F32)
            make_identity(nc, ident[:])
            wpsb = const.tile([11, DM], F32)
            nc.sync.dma_start(out=wpsb[:], in_=wpb[:])
            for t in range(NCL):
                cint = sb.tile([11, LK], F32, tag="cint")
                nc.sync.dma_start(out=cint[:], in_=cin[t])
                for lh in range(2):
                    pcc = ps.tile([114, DM], F32, tag="tp")
                    nc.tensor.matmul(pcc[:], cint[:, lh * 114:lh * 114 + 114],
                                     wpsb[:], start=True, stop=True)
                    gcc = sb.tile([114, DM], F32, tag="gcc")
                    nc.scalar.activation(gcc[:], pcc[:], AF.Gelu)
                    nc.sync.dma_start(out=cc_d[t, lh * 114:lh * 114 + 114, :],
                                      in_=gcc[:])

            cc_flat = cc_d.ap().rearrange("(z a) b c -> z (a b c)", z=1)
            vp_flat = vp.ap().rearrange("(a g) t -> a (g t)", a=1)
            for h in range(H):
                cc2 = sb2.tile([128, 2, DK], F32, tag="cc2")
                for n2 in range(NCL):
                    base = (10 * h + n2) * LK * DK
                    ld = sb.tile([128, 2, DK], F32, tag="ccld")
                    nc.sync.dma_start(out=ld[:, 0, :], in_=cc_flat[
                        0, base:base + 128 * DK].rearrange("(a b) -> a b", b=DK))
                    nc.sync.dma_start(out=ld[0:100, 1, :], in_=cc_flat[
                        0, base + 128 * DK:base + LK * DK].rearrange(
                        "(a b) -> a b", b=DK))
                    if n2 == 0:
                        nc.vector.tensor_copy(cc2[:], ld[:])
                    else:
                        nc.vector.tensor_add(cc2[:], cc2[:], ld[:])
                cc2t = sb2.tile([DK, LK], F32, tag="cc2t")
                pt = ps.tile([DK, 128], F32, tag="tp")
                nc.tensor.transpose(pt[:], cc2[:, 0, :], ident[:, :])
                nc.vector.tensor_copy(cc2t[:, 0:128], pt[:])
                pt2 = ps.tile([DK, 128], F32, tag="tp")
                nc.tensor.transpose(pt2[:, 0:100], cc2[0:100, 1, :],
                                    ident[0:100, 0:100])
                nc.vector.tensor_copy(cc2t[:, 128:LK], pt2[:, 0:100])
                vpa = sb2.tile([128, 2, DK + 1], F32, tag="vpa")
                nc.vector.memset(vpa[:, :, DK:DK + 1], 1.0)
                hb = h * LK * DK
                nc.sync.dma_start(out=vpa[:, 0, 0:DK], in_=vp_flat[
                    0, hb:hb + 128 * DK].rearrange("(a b) -> a b", b=DK))
                nc.sync.dma_start(out=vpa[0:100, 1, 0:DK], in_=vp_flat[
                    0, hb + 128 * DK:hb + LK * DK].rearrange("(a b) -> a b", b=DK))
                qt = sb2.tile([DK, L], F32, tag="qt")
                for i in range(16):
                    qld = sb.tile([128, DK], F32, tag="qld")
                    nc.sync.dma_start(out=qld[:], in_=qin[h, i * 128:(i + 1) * 128, :])
                    qtp = ps.tile([DK, 128], F32, tag="tp")
                    nc.tensor.transpose(qtp[:], qld[:], ident[:, :])
                    nc.vector.tensor_copy(qt[:, i * 128:(i + 1) * 128], qtp[:])
                expt = sb2.tile([128, 2, L], F32, tag="expt")
                for kc in range(2):
                    kl = 128 if kc == 0 else 100
                    for i in range(4):
                        sco = pssco.tile([128, 512], F32, tag="sco")
                        nc.tensor.matmul(sco[0:kl, :],
                                         cc2t[:, kc * 128:kc * 128 + kl],
                                         qt[:, i * 512:(i + 1) * 512],
                                         start=True, stop=True)
                        nc.scalar.activation(expt[0:kl, kc, i * 512:(i + 1) * 512],
                                             sco[0:kl, :], AF.Exp, scale=0.125)
                ctp = psctx.tile([DK + 1, L], F32, tag="ctp")
                for i in range(4):
                    for kc in range(2):
                        kl = 128 if kc == 0 else 100
                        nc.tensor.matmul(ctp[:, i * 512:(i + 1) * 512],
                                         vpa[0:kl, kc, :],
                                         expt[0:kl, kc, i * 512:(i + 1) * 512],
                                         start=(kc == 0), stop=(kc == 1))
                cts = sb2.tile([DK + 1, L], F32, tag="cts")
                nc.vector.tensor_copy(cts[:], ctp[:])
                for i in range(16):
                    cto = ps.tile([128, DK + 1], F32, tag="tp")
                    nc.tensor.transpose(cto[:], cts[:, i * 128:(i + 1) * 128],
                                        ident[0:DK + 1, 0:DK + 1])
                    rcp = sb.tile([128, 1], F32, tag="rcp")
                    nc.vector.reciprocal(rcp[:], cto[:, DK:DK + 1])
                    cfin = sb.tile([128, DK], F32, tag="cfin")
                    nc.vector.tensor_scalar_mul(cfin[:], cto[:, 0:DK], rcp[:])
                    nc.sync.dma_start(out=ctx[h, i * 128:(i + 1) * 128, :],
                                      in_=cfin[:])
    return nc


_CACHE = {}


def kernel(**inputs):
    _patch_tile_drain()
    Q = np.ascontiguousarray(np.asarray(inputs["Q"], np.float32))
    K = np.ascontiguousarray(np.asarray(inputs["K"], np.float32))
    V = np.ascontiguousarray(np.asarray(inputs["V"], np.float32))
    Wq1 = np.asarray(inputs["Wq1"], np.float32)
    bq1 = np.asarray(inputs["bq1"], np.float32)
    Wq2 = np.asarray(inputs["Wq2"], np.float32)
    bq2 = np.asarray(inputs["bq2"], np.float32)
    Wp = np.asarray(inputs["Wp"], np.float32)
    bp = np.asarray(inputs["bp"], np.float32)

    w1pack = np.zeros((128, 175, 40), np.float32)
    for s in range(7):
        for rho in range(25):
            fs, g0 = _slot(s, rho)
            w1pack[g0:g0 + len(fs), s * 25 + rho, :] = Wq1[fs, :]
    w1pack = np.ascontiguousarray(w1pack.reshape(128, 175 * 40))
    w2aug = np.ascontiguousarray(np.vstack([Wq2, bq2[None, :]]), np.float32)
    wpb = np.ascontiguousarray(np.vstack([Wp * 0.01, bp[None, :]]), np.float32)
    bq1c = np.ascontiguousarray(bq1.reshape(40, 1))

    if not _CACHE:
        _CACHE["a"], _CACHE["b"], _CACHE["c"] = _build_a(), _build_b(), _build_c()
    cores = list(range(8))

    # ---- launch A ----
    ra = run_bass_kernel_spmd(
        _CACHE["a"],
        [{"kin": K[b].reshape(G, L), "vin": V[b].reshape(G, L)} for b in range(8)],
        cores).results
    kpt = np.stack([ra[b]["kpt"] for b in range(8)])      # [8, 128, 912]
    vpp = np.stack([ra[b]["vp"] for b in range(8)])       # [8, 512, 228]

    # ---- host glue: per-core KpT[b-s] stack (zeros when b-s < 1) ----
    kjl = np.zeros((8, 128, 7, 912), np.float32)
    for b in range(8):
        for s in range(7):
            j = b - s
            if j >= 1:
                kjl[b, :, s, :] = kpt[j]

    # ---- launch B ----
    rb = run_bass_kernel_spmd(
        _CACHE["b"],
        [{"kjl": kjl[b].reshape(128, 7 * 912), "w1p": w1pack, "w2a": w2aug,
          "bq1": bq1c} for b in range(8)],
        cores).results
    cex = np.stack([rb[b]["cex"] for b in range(8)])      # [8, 10(c), 2280(lk*c2)]
    lsum = sum(float(rb[b]["lossp"][0, 0]) for b in range(8))

    # ---- host glue: per-core centers for its 10 cc blocks ----
    cexr = cex.reshape(8, NCL, LK, NCL)                   # [b, c, lk, i]
    cin = np.zeros((8, NCL, 11, LK), np.float32)
    cin[:, :, 10, :] = 1.0
    for b2 in range(8):
        for t in range(NCL):
            q = 10 * b2 + t
            n, bs = q // 8, q % 8
            cin[b2, t, 0:NCL, :] = cexr[bs, :, :, n]

    # ---- launch C ----
    rc = run_bass_kernel_spmd(
        _CACHE["c"],
        [{"cin": cin[b], "wpb": wpb, "qin": Q[b], "vp": vpp[b]} for b in range(8)],
        cores).results
    context = np.stack([rc[b]["ctx"] for b in range(8)]).reshape(B, H, L, DK)

    loss = np.float32(lsum / 182400.0 + 0.5 * np.log(2 * np.pi)
                      + 22.8 * np.log(228.0))
    return context, loss
